# revision 21
# baseline (speedup 1.0000x reference)
"""Trainium2 Bass kernel for nn_BasisJastrow.

Math (per batch element b):
    J_b = (1/P) * sum_{i<j} chi_j^T C chi_i ,   P = N(N-1)/2, C = coeff.reshape(Nb, Nb)

Device decomposition (per core; data-parallel over the batch axis).
The 48 per-core batches are split into two halves on disjoint SBUF partition
ranges (A = batches 0..23 on partitions 0:64, B = batches 24..47 on 64:128).
All matmul tensors are bf16 (PSUM accumulation stays f32); tolerance is 2e-2.

  layout  Xl[n + 64*half, (b,u)]   n=64 on partitions, 24 batches * 32 per half
  phase 1 S  = Lt.T @ Xl           exclusive prefix sums over particles (PE,
                                   concurrent 64x64 quadrants per half)
  phase 2 Q_p = Xp.T @ Sp          2-batch cross-Gram [64,64] per half; diag
                                   32x32 blocks are G_b, off-diag is garbage
  phase 3 r_p[q] = sum_f Q_p[q,f] * CD2[q,f]  DVE multiplies each 3-pair
                                   bank by the mask (PSUM -> SBUF bf16) and
                                   reduces bank pairs with tensor_reduce
  phase 4 J = id4.T @ R            partition-block reduction of r (PE)

Engine roles: Sync = ring A DMA (half A + const pack) + output DMA.
Scalar = ring B DMA + both S casts (a dummy activation preloads the 1.3us
ACT_TABLE during the DMA window).  GpSimd = Lt/id4 generation (it cannot
access PSUM).  DVE = mask mults, reductions, J copy.  PE = warmup, phases
1/2/4.

Raw Bass (explicit engine blocks + semaphores): the walrus build in this
container rejects any instruction carrying more than one sync wait, which
rules out Tile's generated sem placement; raw Bass emits waits standalone.
"""

import sys

for _p in ("/opt/trn_rl_repo",):
    if _p not in sys.path:
        sys.path.insert(0, _p)

import numpy as np

import concourse.bass as bass
from concourse import mybir
from concourse.bass_utils import run_bass_kernel_spmd

B, N, Nb = 384, 64, 32
NCORES = 8
BS = B // NCORES            # 48 batches per core
HB = BS // 2                # 24 batches per half
NP = HB // 2                # 12 concurrent gram pairs
NPAIR = N * (N - 1) // 2    # 2016
F32 = mybir.dt.float32

MM_DTYPE = "bf16"  # "f32" | "bf16"
USE_BF16 = MM_DTYPE == "bf16"
MM_DT = mybir.dt.bfloat16 if USE_BF16 else mybir.dt.float32

FREE = HB * Nb              # 768 free columns per half
CHUNKS = [384, 384]         # phase-1/cast sub-chunks (NOT DMA chunks)
CHUNK_OFF = [0, 384, 768]
NCHUNK = len(CHUNKS)
NBANK = 4                   # gram psum banks
PPB = 3                     # pairs per bank
N_WARM = 8                  # PE warmup matmuls during the input DMA window
WARM_COLS = 64

CP_COLS = 64                # const pack = CD2 only (Lt, id4 built on-chip)


def build_nc() -> bass.Bass:
    nc = bass.Bass()

    x_d = nc.dram_tensor("x", [128, FREE], MM_DT, kind="ExternalInput")
    cp_d = nc.dram_tensor("cp", [128, CP_COLS], F32, kind="ExternalInput")
    j_d = nc.dram_tensor("j", [4, NP], F32, kind="ExternalOutput")

    from contextlib import ExitStack

    with ExitStack() as ctx:
        ctx.enter_context(
            nc.allow_low_precision("bf16 pair sums stay within the 2e-2 gate")
        )
        x_sb = ctx.enter_context(nc.sbuf_tensor("x_sb", [128, FREE], MM_DT))
        s_sb = ctx.enter_context(nc.sbuf_tensor("s_sb", [128, FREE], MM_DT))
        cp_sb = ctx.enter_context(nc.sbuf_tensor("cp_sb", [128, CP_COLS], F32))
        w_sb = ctx.enter_context(
            nc.sbuf_tensor("w_sb", [128, 4 + WARM_COLS], MM_DT)
        )
        lt_sb = ctx.enter_context(nc.sbuf_tensor("lt_sb", [128, N], MM_DT))
        id4_sb = ctx.enter_context(nc.sbuf_tensor("id4_sb", [128, 4], MM_DT))
        dum_sb = ctx.enter_context(nc.sbuf_tensor("dum_sb", [128, 1], MM_DT))
        e_sb = ctx.enter_context(
            nc.sbuf_tensor("e_sb", [128, NBANK, PPB * 64], MM_DT)
        )
        r_sb = ctx.enter_context(nc.sbuf_tensor("r_sb", [128, NP], MM_DT))
        j_sb = ctx.enter_context(nc.sbuf_tensor("j_sb", [4, NP], F32))
        s_ps = [
            ctx.enter_context(nc.psum_tensor(f"s_ps{c}", [128, CHUNKS[c]], F32))
            for c in range(NCHUNK)
        ]
        # qt0 carries the phase-4 output after its pair columns; qt2/qt3
        # carry the PE warmup scratch.  All stay within one 2KB bank.
        QW = PPB * 64
        WARM_AT = {2: 3, 3: 5}  # bank -> number of warm slots
        q_ps = [
            ctx.enter_context(
                nc.psum_tensor(
                    f"q_ps{k}",
                    [128, QW + 16 if k == 0 else QW + WARM_AT.get(k, 0) * WARM_COLS],
                    F32,
                )
            )
            for k in range(NBANK)
        ]
        dma_a = ctx.enter_context(nc.semaphore("dma_a"))
        dma_b = ctx.enter_context(nc.semaphore("dma_b"))
        dma_c = ctx.enter_context(nc.semaphore("dma_c"))
        dma_o = ctx.enter_context(nc.semaphore("dma_o"))
        pe = ctx.enter_context(nc.semaphore("pe"))
        dve = ctx.enter_context(nc.semaphore("dve"))
        sc = ctx.enter_context(nc.semaphore("sc"))
        gp_w = ctx.enter_context(nc.semaphore("gp_w"))
        block = ctx.enter_context(nc.Block())

        lt = lt_sb[:]
        cd2 = cp_sb[:, 0:CP_COLS]
        cd2r = bass.AP(
            tensor=cd2.tensor,
            offset=cd2.offset,
            ap=[list(cd2.ap[0]), [0, PPB], list(cd2.ap[1])],
        )
        jw = q_ps[0][0:4, QW : QW + NP]

        # pe ledger: phase1 chunks -> 1..NCHUNK; pair p done -> NCHUNK+1+p;
        # phase4 -> NCHUNK+NP+1
        PAIR_DONE = lambda p: NCHUNK + 1 + p
        PH4_DONE = NCHUNK + NP + 1
        # dve ledger: m0,m1 -> 1,2; r01 -> 3; m2,m3 -> 4,5; r23 -> 6; j -> 7
        RED_DONE = 6
        JCOPY_DONE = 7
        # sc ledger: 1,2 = S casts

        @block.sync
        def _(sync):
            # ring A: half A (partitions 0:64) in ONE transfer (DMA cost is
            # per dst-partition line, so column-chunking doubles it), then
            # the const pack (needed only once phase 3 starts)
            sync.dma_start(out=x_sb[0:64, :], in_=x_d[0:64, :]).then_inc(
                dma_a, 16
            )
            sync.dma_start(out=cp_sb[:], in_=cp_d[:]).then_inc(dma_c, 16)

        @block.scalar
        def _(scalar):
            # ring B: half B (partitions 64:128) in one transfer
            scalar.dma_start(out=x_sb[64:128, :], in_=x_d[64:128, :]).then_inc(
                dma_b, 16
            )
            # dummy activation: hides the one-time ACT_TABLE_LOAD (~1.3us)
            # inside the DMA window so the S casts below start immediately
            scalar.wait_ge(gp_w, 1)
            scalar.copy(dum_sb[:], w_sb[:, 0:1])
            # S casts (PSUM f32 -> SBUF bf16) on the activation engine
            for c in range(NCHUNK):
                cs = slice(CHUNK_OFF[c], CHUNK_OFF[c + 1])
                scalar.wait_ge(pe, c + 1)
                scalar.copy(s_sb[:, cs], s_ps[c][:]).then_inc(sc, 1)

        @block.gpsimd
        def _(gpsimd):
            gpsimd.memset(w_sb[:], 1.0).then_inc(gp_w, 1)
            gpsimd.wait_ge(gp_w, 1)
            for h in range(2):
                hs = slice(h * 64, (h + 1) * 64)
                gpsimd.affine_select(
                    out=lt_sb[hs, :],
                    in_=w_sb[hs, 0:N],
                    pattern=[[1, N]],
                    compare_op=mybir.AluOpType.is_gt,
                    fill=0.0,
                    base=0,
                    channel_multiplier=-1,
                ).then_inc(gp_w, 1)
            # id4 for phase 4: col k = ones on partitions 32k:32k+32.
            # Disjoint memsets (the race checker rejects overlapping WAW
            # even on one engine).
            for k in range(4):
                for blk in range(4):
                    inst = gpsimd.memset(
                        id4_sb[32 * blk : 32 * (blk + 1), k : k + 1],
                        1.0 if blk == k else 0.0,
                    )
            inst.then_inc(gp_w, 1)
            # output DMA on the (otherwise idle) software DGE queue; Sync's
            # HWDGE trigger instruction costs ~900ns, this is cheaper
            gpsimd.wait_ge(dve, JCOPY_DONE)
            gpsimd.dma_start(out=j_d[:], in_=j_sb[:]).then_inc(dma_o, 16)

        @block.tensor
        def _(tensor):
            # PE warmup on memset scratch while the input DMAs land
            tensor.wait_ge(gp_w, 1)
            for w in range(N_WARM):
                bank = 3 if w < WARM_AT[3] else 2
                slot = w if w < WARM_AT[3] else w - WARM_AT[3]
                tensor.matmul(
                    q_ps[bank][
                        0:4, QW + slot * WARM_COLS : QW + (slot + 1) * WARM_COLS
                    ],
                    w_sb[0:64, 0:4],
                    w_sb[0:64, 4 : 4 + WARM_COLS],
                    start=True,
                    stop=True,
                )
            tensor.wait_ge(gp_w, 3)
            # phase 1: exclusive prefix sums, concurrent halves
            for c in range(NCHUNK):
                cs = slice(CHUNK_OFF[c], CHUNK_OFF[c + 1])
                if c == 0:
                    tensor.wait_ge(dma_a, 16)
                tensor.matmul(
                    s_ps[c][0:64, :],
                    lt[0:64, :],
                    x_sb[0:64, cs],
                    start=True,
                    stop=True,
                    tile_position=(0, 0),
                )
                if c == 0:
                    tensor.wait_ge(dma_b, 16)
                tensor.matmul(
                    s_ps[c][64:128, :],
                    lt[64:128, :],
                    x_sb[64:128, cs],
                    start=True,
                    stop=True,
                    tile_position=(64, 64),
                ).then_inc(pe, 1)
            # phase 2: 2-batch cross-Grams, concurrent halves
            PPC = NP // NCHUNK
            for p in range(NP):
                ps_ = slice(p * 64, (p + 1) * 64)
                if p == 0:
                    tensor.wait_ge(sc, 1)
                elif p == PPC:
                    tensor.wait_ge(sc, 2)
                q = q_ps[p // PPB][:, (p % PPB) * 64 : (p % PPB + 1) * 64]
                tensor.matmul(
                    q[0:64, :],
                    x_sb[0:64, ps_],
                    s_sb[0:64, ps_],
                    start=True,
                    stop=True,
                    tile_position=(0, 0),
                )
                tensor.matmul(
                    q[64:128, :],
                    x_sb[64:128, ps_],
                    s_sb[64:128, ps_],
                    start=True,
                    stop=True,
                    tile_position=(64, 64),
                ).then_inc(pe, 1)
            # phase 4: partition-block reduction of r columns
            tensor.wait_ge(gp_w, 4)
            tensor.wait_ge(dve, RED_DONE)
            tensor.matmul(
                jw, id4_sb[:], r_sb[:], start=True, stop=True
            ).then_inc(pe, 1)

        @block.vector
        def _(vector):
            # phase 3 mask-mults: e = Q * CD2 per 3-pair bank (PSUM->SBUF),
            # with a fused 2-bank reduce after each pair of mults
            vector.wait_ge(dma_c, 16)
            for g in range(2):
                for k in (2 * g, 2 * g + 1):
                    vector.wait_ge(pe, PAIR_DONE(PPB * k + PPB - 1))
                    vector.tensor_tensor(
                        out=e_sb[:, k].rearrange("p (r f) -> p r f", r=PPB),
                        in0=q_ps[k][:, 0:QW].rearrange("p (r f) -> p r f", r=PPB),
                        in1=cd2r,
                        op=mybir.AluOpType.mult,
                    ).then_inc(dve, 1)
                vector.wait_ge(dve, 3 * g + 2)  # own mults retired
                vector.tensor_reduce(
                    out=r_sb[:, 2 * g * PPB : (2 * g + 2) * PPB],
                    in_=e_sb[:, 2 * g : 2 * g + 2].rearrange(
                        "p b (r f) -> p (b r) f", r=PPB
                    ),
                    axis=mybir.AxisListType.X,
                    op=mybir.AluOpType.add,
                ).then_inc(dve, 1)
            vector.wait_ge(pe, PH4_DONE)
            vector.tensor_copy(j_sb[:], jw).then_inc(dve, 1)

    return nc


def _np_mm_dtype():
    if USE_BF16:
        import ml_dtypes

        return ml_dtypes.bfloat16
    return np.float32


def make_consts(jastrow_coeff: np.ndarray):
    C = np.asarray(jastrow_coeff, dtype=np.float32).reshape(Nb, Nb)
    cp = np.zeros((128, CP_COLS), dtype=np.float32)
    bd2 = np.zeros((64, 64), dtype=np.float32)
    for i in range(2):
        bd2[32 * i : 32 * (i + 1), 32 * i : 32 * (i + 1)] = C / NPAIR
    cp[0:64, :] = bd2
    cp[64:128, :] = bd2
    return cp


def shard_x(basis_single_body: np.ndarray):
    x = np.asarray(basis_single_body, dtype=np.float32)
    xt = np.ascontiguousarray(x.transpose(1, 0, 2))  # [N, B, Nb]
    dt = _np_mm_dtype()
    out = []
    for m in range(NCORES):
        sl = xt[:, m * BS : (m + 1) * BS, :]
        a = sl[:, 0:HB, :].reshape(N, FREE)
        b = sl[:, HB:BS, :].reshape(N, FREE)
        out.append(np.ascontiguousarray(np.concatenate([a, b], axis=0)).astype(dt))
    return out


def unpack_j(j: np.ndarray) -> np.ndarray:
    """j[i, p] -> per-core J[48].

    jw cols 0..5 = tree banks 0/1 (pairs 0..5); cols 6..11 = DVE-reduced
    banks 2/3 (pairs 6..11), so col p is pair p.  blocks 0,1 = half A batch
    2p+i; blocks 2,3 = half B batch 24+2p+(i-2)."""
    j = np.asarray(j, dtype=np.float32)
    ja = j[0:2, :].T.ravel()
    jb = j[2:4, :].T.ravel()
    return np.concatenate([ja, jb]).astype(np.float32)


_NC_CACHE: list = []


def kernel(basis_single_body: np.ndarray, jastrow_coeff: np.ndarray) -> np.ndarray:
    if not _NC_CACHE:
        _NC_CACHE.append(build_nc())
    nc = _NC_CACHE[0]

    cp = make_consts(jastrow_coeff)
    shards = shard_x(basis_single_body)
    in_maps = [{"x": s, "cp": cp} for s in shards]

    res = run_bass_kernel_spmd(nc, in_maps, core_ids=list(range(NCORES)))
    return np.concatenate([unpack_j(np.asarray(r["j"])) for r in res.results])
